# revision 29
# baseline (speedup 1.0000x reference)
"""Trainium2 Bass kernel for nn_DiceLoss_11038065951148.

Reference semantics: cm[t,p] += (t==p)  -> only the diagonal accumulates, so
tp[c] = #{i : pred_i == target_i == c}; fn = fp = 0 exactly.
dice = mean_{c=1..3} 2*tp/(2*tp + 1e-6); loss = balance * (1 - dice**0.75).

Kernel strategy (memory-bound streaming, data-parallel over 8 cores):
  - shard the [1, N] int32 label arrays into 8 contiguous chunks of
    N/8 = 2,097,152 elements, reshaped tile-major [NT, 128, W]
  - per tile: ACT computes t4 = 4*target (bf16); DVE computes
    u = pred + t4 (values 0..15, u == 5c  <=>  pred==target==c), then three
    fused tensor_scalar(is_equal 5c, accum add) ops give per-partition counts
  - tiny [128, NT*3] accumulator tile is DMA'd out; host sums and applies
    the float32 dice formula.
"""

import os
import sys

for _p in ("/opt/trn_rl_repo", "/opt/pypackages"):
    if _p not in sys.path:
        sys.path.insert(0, _p)

import numpy as np

# Set by the last kernel() call when DICE_TRACE=1: the BassKernelResults
# (exec_time_ns etc.) from run_bass_kernel_spmd. Used by test.py only.
last_results = None

N = 16_777_216
NCORES = 8
PER_CORE = N // NCORES  # 2,097,152
P = 128
NT = 4  # tiles per tensor per core
W = PER_CORE // (P * NT)  # 4096


def build(nt=NT, w=W, repeat=1, compute=True):
    import concourse.bacc as bacc
    import concourse.mybir as mybir
    from concourse._compat import axon_active
    from concourse.tile import TileContext

    nc = bacc.Bacc(
        "TRN2",
        target_bir_lowering=False,
        debug=not axon_active(),
        num_devices=NCORES,
        name="dice_hist",
    )
    # pred and target interleaved per tile so each tile is ONE dma (one
    # DMA-sem wait on the consuming compute op — the TT/STT ISA structs have
    # very few sync-wait slots).
    # layout: [nt, P, 2, w] — per partition row, pred then targ, so each
    # tile DMA is one fully-contiguous 2 MB block (16 KB per partition row)
    pt_d = nc.dram_tensor("pt", [nt, P, 2, w], mybir.dt.int32, kind="ExternalInput")
    # rows: 0 = count(u==5) [class1]; 1..3 = sum(sign(u-k)) for
    # k in (9.5, 10.5, 14.5). Host: n2 = (row1-row2)/2, n3 = (row3+N)/2.
    out_d = nc.dram_tensor("out", [4, P, nt], mybir.dt.float32, kind="ExternalOutput")

    THRESH = (9.5, 10.5, 14.5)

    with TileContext(nc) as tc:
        with (
            tc.tile_pool(name="io", bufs=nt) as io_pool,
            tc.tile_pool(name="wk", bufs=(2 if w >= 4096 else 3)) as wk_pool,
            tc.tile_pool(name="acc", bufs=1) as acc_pool,
        ):
            # accumulator tiles, each written by a single engine
            acc1 = acc_pool.tile([P, nt], mybir.dt.float32, tag="acc1")
            accs = [
                acc_pool.tile(
                    [P, nt], mybir.dt.float32, tag=f"accs{k}", name=f"accs{k}"
                )
                for k in range(3)
            ]
            biases = []
            for k, th in enumerate(THRESH):
                b = acc_pool.tile([P, 1], mybir.dt.float32, tag=f"bias{k}")
                nc.gpsimd.memset(b[:], -th)
                biases.append(b)
            for _r in range(repeat):
                for i in range(nt):
                    tile2 = io_pool.tile([P, 2, w], mybir.dt.int32, tag="pt")
                    # alternate between the two HWDGE rings (SP and ACT
                    # sequencers) so DMA completions overlap across rings
                    dma_eng = nc.sync if i % 2 == 0 else nc.scalar
                    dma_eng.dma_start(tile2[:], pt_d[i])
                    if not compute:
                        continue
                    p_v = tile2[:, 0, :]
                    t_v = tile2[:, 1, :]
                    # u = 4*t + p in one fused op; u == 5c  <=>  p == t == c
                    u = wk_pool.tile([P, w], mybir.dt.bfloat16, tag="u")
                    nc.vector.scalar_tensor_tensor(
                        out=u[:],
                        in0=t_v,
                        scalar=4.0,
                        in1=p_v,
                        op0=mybir.AluOpType.mult,
                        op1=mybir.AluOpType.add,
                    )
                    # class 1 on DVE: count(u == 5)
                    dm1 = wk_pool.tile([P, w], mybir.dt.bfloat16, tag="dm1")
                    nc.vector.tensor_scalar(
                        out=dm1[:],
                        in0=u[:],
                        scalar1=5.0,
                        scalar2=None,
                        op0=mybir.AluOpType.is_equal,
                        op1=mybir.AluOpType.add,
                        accum_out=acc1[:, i : i + 1],
                    )
                    # threshold step sums on ACT: sign(u - k) = +/-1 exactly
                    # (u integer, k half-integer), so
                    # sum = 2*count(u > k) - w.
                    for k in range(3):
                        dmk = wk_pool.tile([P, w], mybir.dt.bfloat16, tag="dmA")
                        nc.scalar.activation(
                            out=dmk[:],
                            in_=u[:],
                            func=mybir.ActivationFunctionType.Sign,
                            bias=biases[k][:],
                            scale=1.0,
                            accum_out=accs[k][:, i : i + 1],
                        )
            # gpsimd (SWDGE) for the store-back: it must wait on the other
            # engines, and the HWDGE direct-DMA struct has no wait slots.
            if compute:
                nc.gpsimd.dma_start(out_d[0], acc1[:])
                for k in range(3):
                    nc.gpsimd.dma_start(out_d[k + 1], accs[k][:])
            else:
                nc.gpsimd.dma_start(out_d[:], tile2[:, 0, : 4 * nt])
    nc.compile()
    return nc


_nc_cache = None


def _get_nc():
    global _nc_cache
    if _nc_cache is None:
        _nc_cache = build()
    return _nc_cache


def _dice_from_counts(counts, balance, num_classes):
    # counts: float64 [4]; replicate the reference float32 arithmetic
    tp = counts.astype(np.float32)
    denom = (np.float32(2.0) * tp + np.float32(1e-6)).astype(np.float32)
    dice_per_class = (np.float32(2.0) * tp / denom).astype(np.float32)
    dice = np.float32(dice_per_class[1:].sum()) / np.float32(num_classes - 1)
    loss = np.float32(balance) * (np.float32(1.0) - dice ** np.float32(0.75))
    return np.float32(loss)


def kernel(**inputs):
    pred = np.ascontiguousarray(np.asarray(inputs["pred_labels"], dtype=np.int32))
    targ = np.ascontiguousarray(np.asarray(inputs["target_labels"], dtype=np.int32))
    balance = np.float32(np.asarray(inputs.get("balance", 1.0)))
    num_classes = int(np.asarray(inputs.get("num_classes", 4)))

    from concourse.bass_utils import run_bass_kernel_spmd

    nc = _get_nc()
    pred_sh = pred.reshape(NCORES, NT, P, 1, W)
    targ_sh = targ.reshape(NCORES, NT, P, 1, W)
    # interleave per partition row: [NCORES, NT, P, 2, W]
    pt = np.concatenate([pred_sh, targ_sh], axis=3)
    in_maps = [{"pt": pt[i]} for i in range(NCORES)]
    trace = os.environ.get("DICE_TRACE", "") == "1"
    res = run_bass_kernel_spmd(
        nc, in_maps, core_ids=list(range(NCORES)), trace=trace
    )
    global last_results
    last_results = res

    counts = np.zeros(4, dtype=np.float64)
    for r in res.results:
        a = np.asarray(r["out"], dtype=np.float64).sum(axis=(1, 2))  # [4]
        counts[1] += a[0]
        counts[2] += (a[1] - a[2]) / 2.0
        counts[3] += (a[3] + PER_CORE) / 2.0
    counts = np.rint(counts)
    return _dice_from_counts(counts, balance, num_classes)


# revision 39
# speedup vs baseline: 1.0336x; 1.0336x over previous
"""Trainium2 Bass kernel for nn_DiceLoss_11038065951148.

Reference semantics: cm[t,p] += (t==p)  -> only the diagonal accumulates, so
tp[c] = #{i : pred_i == target_i == c}; fn = fp = 0 exactly.
dice = mean_{c=1..3} 2*tp/(2*tp + 1e-6); loss = balance * (1 - dice**0.75).

Kernel strategy (memory-bound streaming, data-parallel over 8 cores):
  - shard the [1, N] int32 label arrays into 8 contiguous chunks of
    N/8 = 2,097,152 elements, reshaped tile-major [NT, 128, W]
  - per tile: ACT computes t4 = 4*target (bf16); DVE computes
    u = pred + t4 (values 0..15, u == 5c  <=>  pred==target==c), then three
    fused tensor_scalar(is_equal 5c, accum add) ops give per-partition counts
  - tiny [128, NT*3] accumulator tile is DMA'd out; host sums and applies
    the float32 dice formula.
"""

import os
import sys

for _p in ("/opt/trn_rl_repo", "/opt/pypackages"):
    if _p not in sys.path:
        sys.path.insert(0, _p)

import numpy as np

# Set by the last kernel() call when DICE_TRACE=1: the BassKernelResults
# (exec_time_ns etc.) from run_bass_kernel_spmd. Used by test.py only.
last_results = None

N = 16_777_216
NCORES = 8
PER_CORE = N // NCORES  # 2,097,152
P = 128
TOT = PER_CORE // P  # 16384 elements per partition per tensor
NT = 4  # tiles per tensor per core (uniform default)
W = TOT // NT  # 4096
# ramped schedule: small tiles first (compute starts early) and last
# (short tail), 4 MB tiles in the middle for bandwidth
WIDTHS_RAMP = (1024, 1024, 1024, 4096, 4096, 4096, 1024)


def build(nt=NT, w=W, repeat=1, compute=True, widths=None, serialize=False):
    import concourse.bacc as bacc
    import concourse.mybir as mybir
    from concourse._compat import axon_active
    from concourse.tile import TileContext, add_dep_helper

    nc = bacc.Bacc(
        "TRN2",
        target_bir_lowering=False,
        debug=not axon_active(),
        num_devices=NCORES,
        name="dice_hist",
    )
    if widths is None:
        widths = [w] * nt
    widths = list(widths)
    tot = sum(widths)
    nt = len(widths)
    offs = [sum(widths[:i]) for i in range(nt)]
    # pred and target interleaved per partition row so each tile is ONE dma
    # (one DMA-sem wait on the consuming compute op — the compute ISA structs
    # have very few sync-wait slots).
    # layout: [P, 2, tot]; tile i = columns [offs[i], offs[i]+widths[i])
    pt_d = nc.dram_tensor("pt", [P, 2, tot], mybir.dt.int32, kind="ExternalInput")
    # rows: 0 = count(u==5) [class1]; 1..3 = sum(sign(u-k)) for
    # k in (9.5, 10.5, 14.5). Host: n2 = (row1-row2)/2, n3 = (row3+N)/2.
    out_d = nc.dram_tensor("out", [4, P, nt], mybir.dt.float32, kind="ExternalOutput")

    THRESH = (9.5, 10.5, 14.5)
    n_of_width = {wd: widths.count(wd) for wd in set(widths)}

    with TileContext(nc) as tc:
        with (
            tc.tile_pool(name="io", bufs=1) as io_pool,
            tc.tile_pool(name="wk", bufs=2) as wk_pool,
            tc.tile_pool(name="acc", bufs=1) as acc_pool,
        ):
            # accumulator tiles, each written by a single engine
            acc1 = acc_pool.tile([P, nt], mybir.dt.float32, tag="acc1")
            accs = [
                acc_pool.tile(
                    [P, nt], mybir.dt.float32, tag=f"accs{k}", name=f"accs{k}"
                )
                for k in range(3)
            ]
            biases = []
            for k, th in enumerate(THRESH):
                b = acc_pool.tile([P, 1], mybir.dt.float32, tag=f"bias{k}")
                nc.gpsimd.memset(b[:], -th)
                biases.append(b)
            prev_tail = None
            for _r in range(repeat):
                tail_inst = None
                for i in range(nt):
                    wd = widths[i]
                    tile2 = io_pool.tile(
                        [P, 2, wd],
                        mybir.dt.int32,
                        tag=f"pt{wd}",
                        bufs=min(n_of_width[wd], 3),
                    )
                    d = nc.sync.dma_start(
                        tile2[:], pt_d[:, :, offs[i] : offs[i] + wd]
                    )
                    if serialize and prev_tail is not None:
                        add_dep_helper(
                            d.ins, prev_tail, sync=True, reason="serialize repeats"
                        )
                    if not compute:
                        continue
                    p_v = tile2[:, 0, :]
                    t_v = tile2[:, 1, :]
                    # u = 4*t + p in one fused op; u == 5c  <=>  p == t == c
                    u = wk_pool.tile([P, wd], mybir.dt.bfloat16, tag=f"u{wd}")
                    nc.vector.scalar_tensor_tensor(
                        out=u[:],
                        in0=t_v,
                        scalar=4.0,
                        in1=p_v,
                        op0=mybir.AluOpType.mult,
                        op1=mybir.AluOpType.add,
                    )
                    # class 1 on DVE: count(u == 5)
                    dm1 = wk_pool.tile([P, wd], mybir.dt.bfloat16, tag=f"dm1{wd}")
                    nc.vector.tensor_scalar(
                        out=dm1[:],
                        in0=u[:],
                        scalar1=5.0,
                        scalar2=None,
                        op0=mybir.AluOpType.is_equal,
                        op1=mybir.AluOpType.add,
                        accum_out=acc1[:, i : i + 1],
                    )
                    # threshold step sums on ACT: sign(u - k) = +/-1 exactly
                    # (u integer, k half-integer), so
                    # sum = 2*count(u > k) - w.
                    for k in range(3):
                        dmk = wk_pool.tile([P, wd], mybir.dt.bfloat16, tag=f"dmA{wd}")
                        a = nc.scalar.activation(
                            out=dmk[:],
                            in_=u[:],
                            func=mybir.ActivationFunctionType.Sign,
                            bias=biases[k][:],
                            scale=1.0,
                            accum_out=accs[k][:, i : i + 1],
                        )
                        tail_inst = a.ins
                prev_tail = tail_inst
            # gpsimd (SWDGE) for the store-back: it must wait on the other
            # engines, and the HWDGE direct-DMA struct has no wait slots.
            if compute:
                nc.gpsimd.dma_start(out_d[0], acc1[:])
                for k in range(3):
                    nc.gpsimd.dma_start(out_d[k + 1], accs[k][:])
            else:
                nc.gpsimd.dma_start(out_d[:], tile2[:, 0, : 4 * nt])
    nc.compile()
    return nc


_nc_cache = None


def _get_nc():
    global _nc_cache
    if _nc_cache is None:
        _nc_cache = build()
    return _nc_cache


def _dice_from_counts(counts, balance, num_classes):
    # counts: float64 [4]; replicate the reference float32 arithmetic
    tp = counts.astype(np.float32)
    denom = (np.float32(2.0) * tp + np.float32(1e-6)).astype(np.float32)
    dice_per_class = (np.float32(2.0) * tp / denom).astype(np.float32)
    dice = np.float32(dice_per_class[1:].sum()) / np.float32(num_classes - 1)
    loss = np.float32(balance) * (np.float32(1.0) - dice ** np.float32(0.75))
    return np.float32(loss)


def kernel(**inputs):
    pred = np.ascontiguousarray(np.asarray(inputs["pred_labels"], dtype=np.int32))
    targ = np.ascontiguousarray(np.asarray(inputs["target_labels"], dtype=np.int32))
    balance = np.float32(np.asarray(inputs.get("balance", 1.0)))
    num_classes = int(np.asarray(inputs.get("num_classes", 4)))

    from concourse.bass_utils import run_bass_kernel_spmd

    nc = _get_nc()
    pred_sh = pred.reshape(NCORES, P, 1, TOT)
    targ_sh = targ.reshape(NCORES, P, 1, TOT)
    # interleave per partition row: [NCORES, P, 2, TOT]
    pt = np.concatenate([pred_sh, targ_sh], axis=2)
    in_maps = [{"pt": pt[i]} for i in range(NCORES)]
    trace = os.environ.get("DICE_TRACE", "") == "1"
    res = run_bass_kernel_spmd(
        nc, in_maps, core_ids=list(range(NCORES)), trace=trace
    )
    global last_results
    last_results = res

    counts = np.zeros(4, dtype=np.float64)
    for r in res.results:
        a = np.asarray(r["out"], dtype=np.float64).sum(axis=(1, 2))  # [4]
        counts[1] += a[0]
        counts[2] += (a[1] - a[2]) / 2.0
        counts[3] += (a[3] + PER_CORE) / 2.0
    counts = np.rint(counts)
    return _dice_from_counts(counts, balance, num_classes)


# revision 42
# speedup vs baseline: 1.0747x; 1.0398x over previous
"""Trainium2 Bass kernel for nn_DiceLoss_11038065951148.

Reference semantics: cm[t,p] += (t==p)  -> only the diagonal accumulates, so
tp[c] = #{i : pred_i == target_i == c}; fn = fp = 0 exactly.
dice = mean_{c=1..3} 2*tp/(2*tp + 1e-6); loss = balance * (1 - dice**0.75).

Kernel strategy (memory-bound streaming, data-parallel over 8 cores):
  - shard the [1, N] int32 label arrays into 8 contiguous chunks of
    N/8 = 2,097,152 elements, reshaped tile-major [NT, 128, W]
  - per tile: ACT computes t4 = 4*target (bf16); DVE computes
    u = pred + t4 (values 0..15, u == 5c  <=>  pred==target==c), then three
    fused tensor_scalar(is_equal 5c, accum add) ops give per-partition counts
  - tiny [128, NT*3] accumulator tile is DMA'd out; host sums and applies
    the float32 dice formula.
"""

import os
import sys

for _p in ("/opt/trn_rl_repo", "/opt/pypackages"):
    if _p not in sys.path:
        sys.path.insert(0, _p)

import numpy as np

# Set by the last kernel() call when DICE_TRACE=1: the BassKernelResults
# (exec_time_ns etc.) from run_bass_kernel_spmd. Used by test.py only.
last_results = None

N = 16_777_216
NCORES = 8
PER_CORE = N // NCORES  # 2,097,152
P = 128
TOT = PER_CORE // P  # 16384 elements per partition per tensor
NT = 4  # tiles per tensor per core (uniform default)
W = TOT // NT  # 4096
# ramped schedule: small tiles first (compute starts early) and last
# (short tail), 4 MB tiles in the middle for bandwidth
WIDTHS_RAMP = (1024, 1024, 1024, 4096, 4096, 4096, 1024)


def build(nt=NT, w=W, repeat=1, compute=True, widths=None, serialize=False):
    import concourse.bacc as bacc
    import concourse.mybir as mybir
    from concourse._compat import axon_active
    from concourse.tile import TileContext, add_dep_helper

    nc = bacc.Bacc(
        "TRN2",
        target_bir_lowering=False,
        debug=not axon_active(),
        num_devices=NCORES,
        name="dice_hist",
    )
    if widths is None:
        widths = [w] * nt
    widths = list(widths)
    tot = sum(widths)
    nt = len(widths)
    offs = [sum(widths[:i]) for i in range(nt)]
    # pred and target interleaved per partition row so each tile is ONE dma
    # (one DMA-sem wait on the consuming compute op — the compute ISA structs
    # have very few sync-wait slots).
    # layout: [P, 2, tot]; tile i = columns [offs[i], offs[i]+widths[i])
    pt_d = nc.dram_tensor("pt", [P, 2, tot], mybir.dt.int32, kind="ExternalInput")
    # rows (middle axis): 0 = count(u==5) [class1]; 1..3 = sum(sign(u-k))
    # for k in (9.5, 10.5, 14.5). Host: n2 = (row1-row2)/2, n3 = (row3+N)/2.
    out_d = nc.dram_tensor("out", [P, 4, nt], mybir.dt.float32, kind="ExternalOutput")

    THRESH = (9.5, 10.5, 14.5)
    n_of_width = {wd: widths.count(wd) for wd in set(widths)}

    with TileContext(nc) as tc:
        with (
            tc.tile_pool(name="io", bufs=1) as io_pool,
            tc.tile_pool(name="wk", bufs=2) as wk_pool,
            tc.tile_pool(name="acc", bufs=1) as acc_pool,
        ):
            # one accumulator tile; row 0 written by DVE, rows 1-3 by ACT
            # (disjoint slices, so no cross-engine hazards)
            acc_all = acc_pool.tile([P, 4, nt], mybir.dt.float32, tag="acc")
            acc1 = acc_all[:, 0, :]
            accs = [acc_all[:, k + 1, :] for k in range(3)]
            biases = []
            for k, th in enumerate(THRESH):
                b = acc_pool.tile([P, 1], mybir.dt.float32, tag=f"bias{k}")
                nc.gpsimd.memset(b[:], -th)
                biases.append(b)
            prev_tail = None
            for _r in range(repeat):
                tail_inst = None
                for i in range(nt):
                    wd = widths[i]
                    tile2 = io_pool.tile(
                        [P, 2, wd],
                        mybir.dt.int32,
                        tag=f"pt{wd}",
                        bufs=min(n_of_width[wd], 3),
                    )
                    d = nc.sync.dma_start(
                        tile2[:], pt_d[:, :, offs[i] : offs[i] + wd]
                    )
                    if serialize and prev_tail is not None:
                        add_dep_helper(
                            d.ins, prev_tail, sync=True, reason="serialize repeats"
                        )
                    if not compute:
                        continue
                    p_v = tile2[:, 0, :]
                    t_v = tile2[:, 1, :]
                    # u = 4*t + p in one fused op; u == 5c  <=>  p == t == c
                    u = wk_pool.tile([P, wd], mybir.dt.bfloat16, tag=f"u{wd}")
                    nc.vector.scalar_tensor_tensor(
                        out=u[:],
                        in0=t_v,
                        scalar=4.0,
                        in1=p_v,
                        op0=mybir.AluOpType.mult,
                        op1=mybir.AluOpType.add,
                    )
                    # class 1 on DVE: count(u == 5)
                    dm1 = wk_pool.tile([P, wd], mybir.dt.bfloat16, tag=f"dm1{wd}")
                    nc.vector.tensor_scalar(
                        out=dm1[:],
                        in0=u[:],
                        scalar1=5.0,
                        scalar2=None,
                        op0=mybir.AluOpType.is_equal,
                        op1=mybir.AluOpType.add,
                        accum_out=acc1[:, i : i + 1],
                    )
                    # threshold step sums on ACT: sign(u - k) = +/-1 exactly
                    # (u integer, k half-integer), so
                    # sum = 2*count(u > k) - w.
                    for k in range(3):
                        dmk = wk_pool.tile([P, wd], mybir.dt.bfloat16, tag=f"dmA{wd}")
                        a = nc.scalar.activation(
                            out=dmk[:],
                            in_=u[:],
                            func=mybir.ActivationFunctionType.Sign,
                            bias=biases[k][:],
                            scale=1.0,
                            accum_out=accs[k][:, i : i + 1],
                        )
                        tail_inst = a.ins
                prev_tail = tail_inst
            if compute:
                nc.sync.dma_start(out_d[:], acc_all[:])
            else:
                nc.gpsimd.dma_start(out_d[:], tile2[:, 0, : 4 * nt])
    nc.compile()
    return nc


_nc_cache = None


def _get_nc():
    global _nc_cache
    if _nc_cache is None:
        _nc_cache = build()
    return _nc_cache


def _dice_from_counts(counts, balance, num_classes):
    # counts: float64 [4]; replicate the reference float32 arithmetic
    tp = counts.astype(np.float32)
    denom = (np.float32(2.0) * tp + np.float32(1e-6)).astype(np.float32)
    dice_per_class = (np.float32(2.0) * tp / denom).astype(np.float32)
    dice = np.float32(dice_per_class[1:].sum()) / np.float32(num_classes - 1)
    loss = np.float32(balance) * (np.float32(1.0) - dice ** np.float32(0.75))
    return np.float32(loss)


def kernel(**inputs):
    pred = np.ascontiguousarray(np.asarray(inputs["pred_labels"], dtype=np.int32))
    targ = np.ascontiguousarray(np.asarray(inputs["target_labels"], dtype=np.int32))
    balance = np.float32(np.asarray(inputs.get("balance", 1.0)))
    num_classes = int(np.asarray(inputs.get("num_classes", 4)))

    from concourse.bass_utils import run_bass_kernel_spmd

    nc = _get_nc()
    pred_sh = pred.reshape(NCORES, P, 1, TOT)
    targ_sh = targ.reshape(NCORES, P, 1, TOT)
    # interleave per partition row: [NCORES, P, 2, TOT]
    pt = np.concatenate([pred_sh, targ_sh], axis=2)
    in_maps = [{"pt": pt[i]} for i in range(NCORES)]
    trace = os.environ.get("DICE_TRACE", "") == "1"
    res = run_bass_kernel_spmd(
        nc, in_maps, core_ids=list(range(NCORES)), trace=trace
    )
    global last_results
    last_results = res

    counts = np.zeros(4, dtype=np.float64)
    for r in res.results:
        a = np.asarray(r["out"], dtype=np.float64).sum(axis=(1, 2))  # [4]
        counts[1] += a[0]
        counts[2] += (a[1] - a[2]) / 2.0
        counts[3] += (a[3] + PER_CORE) / 2.0
    counts = np.rint(counts)
    return _dice_from_counts(counts, balance, num_classes)


# revision 45
# speedup vs baseline: 1.7165x; 1.5971x over previous
"""Trainium2 Bass kernel for nn_DiceLoss_11038065951148.

Reference semantics: cm[t,p] += (t==p)  -> only the diagonal accumulates, so
tp[c] = #{i : pred_i == target_i == c}; fn = fp = 0 exactly.
dice = mean_{c=1..3} 2*tp/(2*tp + 1e-6); loss = balance * (1 - dice**0.75).

Kernel strategy (memory-bound streaming, data-parallel over 8 cores):
  - shard the [1, N] int32 label arrays into 8 contiguous chunks of
    N/8 = 2,097,152 elements, reshaped tile-major [NT, 128, W]
  - per tile: ACT computes t4 = 4*target (bf16); DVE computes
    u = pred + t4 (values 0..15, u == 5c  <=>  pred==target==c), then three
    fused tensor_scalar(is_equal 5c, accum add) ops give per-partition counts
  - tiny [128, NT*3] accumulator tile is DMA'd out; host sums and applies
    the float32 dice formula.
"""

import os
import sys

for _p in ("/opt/trn_rl_repo", "/opt/pypackages"):
    if _p not in sys.path:
        sys.path.insert(0, _p)

import numpy as np

# Set by the last kernel() call when DICE_TRACE=1: the BassKernelResults
# (exec_time_ns etc.) from run_bass_kernel_spmd. Used by test.py only.
last_results = None

N = 16_777_216
NCORES = 8
PER_CORE = N // NCORES  # 2,097,152
P = 128
TOT = PER_CORE // P  # 16384 elements per partition per tensor
NT = 4  # tiles per tensor per core (uniform default)
W = TOT // NT  # 4096
# ramped schedule: small tiles first (compute starts early) and last
# (short tail), 4 MB tiles in the middle for bandwidth
WIDTHS_RAMP = (1024, 1024, 1024, 4096, 4096, 4096, 1024)
WIDTHS_RAMP2 = (1024, 1024, 2048, 2048, 2048, 2048, 2048, 2048, 1024, 1024)


def build(nt=NT, w=W, repeat=1, compute=True, widths=None, serialize=False):
    import concourse.bacc as bacc
    import concourse.mybir as mybir
    from concourse._compat import axon_active
    from concourse.tile import TileContext, add_dep_helper

    nc = bacc.Bacc(
        "TRN2",
        target_bir_lowering=False,
        debug=not axon_active(),
        num_devices=NCORES,
        name="dice_hist",
    )
    if widths is None:
        widths = [w] * nt
    widths = list(widths)
    tot = sum(widths)
    nt = len(widths)
    offs = [sum(widths[:i]) for i in range(nt)]
    # pred and target interleaved per partition row so each tile is ONE dma
    # (one DMA-sem wait on the consuming compute op — the compute ISA structs
    # have very few sync-wait slots).
    # layout: [P, 2, tot]; tile i = columns [offs[i], offs[i]+widths[i])
    pt_d = nc.dram_tensor("pt", [P, 2, tot], mybir.dt.int32, kind="ExternalInput")
    # rows (middle axis): 0 = count(u==5) [class1]; 1..3 = sum(sign(u-k))
    # for k in (9.5, 10.5, 14.5). Host: n2 = (row1-row2)/2, n3 = (row3+N)/2.
    out_d = nc.dram_tensor("out", [P, 4, nt], mybir.dt.float32, kind="ExternalOutput")

    THRESH = (9.5, 10.5, 14.5)
    n_of_width = {wd: widths.count(wd) for wd in set(widths)}

    with TileContext(nc) as tc:
        with (
            tc.tile_pool(name="io", bufs=1) as io_pool,
            tc.tile_pool(name="wk", bufs=2) as wk_pool,
            tc.tile_pool(name="acc", bufs=1) as acc_pool,
        ):
            # one accumulator tile; row 0 written by DVE, rows 1-3 by ACT
            # (disjoint slices, so no cross-engine hazards)
            acc_all = acc_pool.tile([P, 4, nt], mybir.dt.float32, tag="acc")
            acc1 = acc_all[:, 0, :]
            accs = [acc_all[:, k + 1, :] for k in range(3)]
            biases = []
            for k, th in enumerate(THRESH):
                b = acc_pool.tile([P, 1], mybir.dt.float32, tag=f"bias{k}")
                nc.gpsimd.memset(b[:], -th)
                biases.append(b)
            prev_tail = None
            for _r in range(repeat):
                tail_inst = None
                for i in range(nt):
                    wd = widths[i]
                    tile2 = io_pool.tile(
                        [P, 2, wd],
                        mybir.dt.int32,
                        tag=f"pt{wd}",
                        bufs=min(n_of_width[wd], 3),
                    )
                    d = nc.sync.dma_start(
                        tile2[:], pt_d[:, :, offs[i] : offs[i] + wd]
                    )
                    if serialize and prev_tail is not None:
                        add_dep_helper(
                            d.ins, prev_tail, sync=True, reason="serialize repeats"
                        )
                    if not compute:
                        continue
                    p_v = tile2[:, 0, :]
                    t_v = tile2[:, 1, :]
                    # u = 4*t + p in one fused op; u == 5c  <=>  p == t == c
                    u = wk_pool.tile([P, wd], mybir.dt.bfloat16, tag=f"u{wd}")
                    nc.vector.scalar_tensor_tensor(
                        out=u[:],
                        in0=t_v,
                        scalar=4.0,
                        in1=p_v,
                        op0=mybir.AluOpType.mult,
                        op1=mybir.AluOpType.add,
                    )
                    # class 1 on DVE: count(u == 5)
                    dm1 = wk_pool.tile([P, wd], mybir.dt.bfloat16, tag=f"dm1{wd}")
                    nc.vector.tensor_scalar(
                        out=dm1[:],
                        in0=u[:],
                        scalar1=5.0,
                        scalar2=None,
                        op0=mybir.AluOpType.is_equal,
                        op1=mybir.AluOpType.add,
                        accum_out=acc1[:, i : i + 1],
                    )
                    # threshold step sums on ACT: sign(u - k) = +/-1 exactly
                    # (u integer, k half-integer), so
                    # sum = 2*count(u > k) - w.
                    for k in range(3):
                        dmk = wk_pool.tile([P, wd], mybir.dt.bfloat16, tag=f"dmA{wd}")
                        a = nc.scalar.activation(
                            out=dmk[:],
                            in_=u[:],
                            func=mybir.ActivationFunctionType.Sign,
                            bias=biases[k][:],
                            scale=1.0,
                            accum_out=accs[k][:, i : i + 1],
                        )
                        tail_inst = a.ins
                prev_tail = tail_inst
            if compute:
                nc.sync.dma_start(out_d[:], acc_all[:])
            else:
                nc.gpsimd.dma_start(out_d[:], tile2[:, 0, : 4 * nt])
    nc.compile()
    return nc


_nc_cache = None


def _get_nc():
    global _nc_cache
    if _nc_cache is None:
        _nc_cache = build(widths=WIDTHS_RAMP2)
    return _nc_cache


def _dice_from_counts(counts, balance, num_classes):
    # counts: float64 [4]; replicate the reference float32 arithmetic
    tp = counts.astype(np.float32)
    denom = (np.float32(2.0) * tp + np.float32(1e-6)).astype(np.float32)
    dice_per_class = (np.float32(2.0) * tp / denom).astype(np.float32)
    dice = np.float32(dice_per_class[1:].sum()) / np.float32(num_classes - 1)
    loss = np.float32(balance) * (np.float32(1.0) - dice ** np.float32(0.75))
    return np.float32(loss)


def kernel(**inputs):
    pred = np.ascontiguousarray(np.asarray(inputs["pred_labels"], dtype=np.int32))
    targ = np.ascontiguousarray(np.asarray(inputs["target_labels"], dtype=np.int32))
    balance = np.float32(np.asarray(inputs.get("balance", 1.0)))
    num_classes = int(np.asarray(inputs.get("num_classes", 4)))

    from concourse.bass_utils import run_bass_kernel_spmd

    nc = _get_nc()
    pred_sh = pred.reshape(NCORES, P, 1, TOT)
    targ_sh = targ.reshape(NCORES, P, 1, TOT)
    # interleave per partition row: [NCORES, P, 2, TOT]
    pt = np.concatenate([pred_sh, targ_sh], axis=2)
    in_maps = [{"pt": pt[i]} for i in range(NCORES)]
    trace = os.environ.get("DICE_TRACE", "") == "1"
    res = run_bass_kernel_spmd(
        nc, in_maps, core_ids=list(range(NCORES)), trace=trace
    )
    global last_results
    last_results = res

    counts = np.zeros(4, dtype=np.float64)
    for r in res.results:
        a = np.asarray(r["out"], dtype=np.float64).sum(axis=(0, 2))  # [4]
        counts[1] += a[0]
        counts[2] += (a[1] - a[2]) / 2.0
        counts[3] += (a[3] + PER_CORE) / 2.0
    counts = np.rint(counts)
    return _dice_from_counts(counts, balance, num_classes)
